# revision 1
# baseline (speedup 1.0000x reference)
"""MultiHeadSimilarity kernel for 8 Trainium2 NeuronCores.

Reference computation (per batch b):
    Q = wq @ x[b];  K = wk @ y[b]                       (channel-mixing matmuls)
    per head h (d=64):  A = relu(Qh^T Kh) * scale, masked by xy_mask
    C = A @ Kh^T, normalized per-row by 1/max(sum(mask, y), 1)
    out = wo @ (0.5 * (Q + C))

Sharding: data-parallel over batch; 16 batches / 8 cores = 2 per core.
Weights replicated. No cross-core communication.

Device algorithm (per core, fp16 compute with fp32 PSUM accumulation):
  - Q = wqT.T @ x, K = wkT.T @ y, KT = y.T @ wkT (natural-layout matmuls; the
    K transpose needed by the C-contraction is computed as a second projection
    instead of an on-chip transpose).
  - A is computed transposed (y on partitions) per head; relu+mask are fused
    into one DVE scalar_tensor_tensor: (A max 0) * maskT. Two heads are packed
    in the 128-wide PE array (K=64 row groups / M=64 col groups).
  - n_el row counts come from a ones^T @ maskT matmul; inv = 1/(8*max(n,1))
    folds the 1/sqrt(d) attention scale; 0.5 is folded into woT on the host.
"""
import sys

if "/opt/trn_rl_repo" not in sys.path:
    sys.path.insert(0, "/opt/trn_rl_repo")

import numpy as np

import concourse.tile as tile
from concourse import bacc, mybir
from concourse.bass_utils import run_bass_kernel_spmd

F16 = mybir.dt.float16
F32 = mybir.dt.float32
U8 = mybir.dt.uint8
AL = mybir.AluOpType

N_CORES = 8
B, U, LX, LY, H, D = 16, 512, 1024, 1024, 8, 64
BPC = B // N_CORES          # batches per core
KB = U // 128               # 4  k-tiles over channels
HP = H // 2                 # 4  head pairs
YT = LY // 128              # 8  y tiles
XH = LX // 512              # 2  x halves
INV_SCALE = float(D) ** 0.5  # 8.0; attention scale = 1/8

# fraction of relu+mask tiles routed through ScalarE (relu) + DVE (mul)
# instead of the fused DVE scalar_tensor_tensor; tunes DVE/ACT balance.
ACT_PCT = 50

TRACE = False
_CACHE = {}


def _build():
    nc = bacc.Bacc("TRN2", target_bir_lowering=False, debug=False,
                   num_devices=N_CORES)
    x_e = nc.dram_tensor("x", [BPC, U, LX], F16, kind="ExternalInput")
    y_e = nc.dram_tensor("y", [BPC, U, LY], F16, kind="ExternalInput")
    mt_e = nc.dram_tensor("mt", [BPC, LY, LX], U8, kind="ExternalInput")
    wqt_e = nc.dram_tensor("wqt", [U, U], F16, kind="ExternalInput")
    wkt_e = nc.dram_tensor("wkt", [U, U], F16, kind="ExternalInput")
    wot_e = nc.dram_tensor("wot", [U, U], F16, kind="ExternalInput")
    o_e = nc.dram_tensor("o", [BPC, U, LX], F32, kind="ExternalOutput")

    with tile.TileContext(nc) as tc:
        _emit(nc, tc, x_e, y_e, mt_e, wqt_e, wkt_e, wot_e, o_e)
    nc.compile()
    return nc


def _emit(nc, tc, x_e, y_e, mt_e, wqt_e, wkt_e, wot_e, o_e):
    import contextlib
    ctx = contextlib.ExitStack()
    with ctx:
        wp = ctx.enter_context(tc.tile_pool(name="wp", bufs=1))
        io = ctx.enter_context(tc.tile_pool(name="io", bufs=2))
        pr = ctx.enter_context(tc.tile_pool(name="pr", bufs=1))
        amp = ctx.enter_context(tc.tile_pool(name="amp", bufs=2))
        osp = ctx.enter_context(tc.tile_pool(name="osp", bufs=4))
        pa = ctx.enter_context(tc.tile_pool(name="pa", bufs=2, space="PSUM"))
        pc = ctx.enter_context(tc.tile_pool(name="pc", bufs=1, space="PSUM"))

        # weights, loaded once
        WQT = wp.tile([128, KB, U], F16, tag="wqt")
        WKT = wp.tile([128, KB, U], F16, tag="wkt")
        WOT = wp.tile([128, KB, U], F16, tag="wot")
        for w_t, w_e in ((WQT, wqt_e), (WKT, wkt_e), (WOT, wot_e)):
            nc.sync.dma_start(w_t[:], w_e.ap().rearrange("(k p) o -> p k o", p=128))
        ones = wp.tile([128, 1], F16, tag="ones")
        nc.vector.memset(ones[:], 1.0)

        for b in range(BPC):
            # ---- input loads ----
            X = io.tile([128, KB, LX], F16, tag="x", name=f"x{b}")
            nc.sync.dma_start(X[:], x_e.ap()[b].rearrange("(k p) l -> p k l", p=128))
            Y = io.tile([128, KB, LY], F16, tag="y", name=f"y{b}")
            nc.sync.dma_start(Y[:], y_e.ap()[b].rearrange("(k p) l -> p k l", p=128))
            MT = io.tile([128, YT, LX], U8, tag="mt", name=f"mt{b}")
            nc.sync.dma_start(MT[:], mt_e.ap()[b].rearrange("(t p) l -> p t l", p=128))

            # ---- projections ----
            Q = pr.tile([128, KB, LX], F16, tag="q", name=f"q{b}")
            K = pr.tile([128, KB, LY], F16, tag="k", name=f"k{b}")
            for w_t, src, dst, nfree in ((WQT, X, Q, LX), (WKT, Y, K, LY)):
                for m in range(KB):
                    ps = pa.tile([128, 1024], F32, tag="a", name=f"pj{b}_{dst.name}_{m}")
                    for k in range(KB):
                        for n in range(nfree // 512):
                            nc.tensor.matmul(
                                ps[:, n * 512:(n + 1) * 512],
                                w_t[:, k, m * 128:(m + 1) * 128],
                                src[:, k, n * 512:(n + 1) * 512],
                                start=(k == 0), stop=(k == KB - 1))
                    nc.scalar.copy(dst[:, m, :], ps[:])
            KT = pr.tile([128, YT, U], F16, tag="kt", name=f"kt{b}")
            for lt in range(YT):
                ps = pa.tile([128, 1024], F32, tag="a", name=f"pkt{b}_{lt}")
                for k in range(KB):
                    nc.tensor.matmul(ps[:, 0:512],
                                     Y[:, k, lt * 128:(lt + 1) * 128],
                                     WKT[:, k, :512],
                                     start=(k == 0), stop=(k == KB - 1))
                nc.scalar.copy(KT[:, lt, :], ps[:, 0:512])

            # ---- mask: cast, row counts, inverse ----
            MTF = pr.tile([128, YT, LX], F16, tag="mtf", name=f"mtf{b}")
            for yt in range(YT):
                nc.gpsimd.tensor_copy(MTF[:, yt, :], MT[:, yt, :])
            nel = pa.tile([1, 1024], F32, tag="a", name=f"nel{b}")
            for xh in range(XH):
                for yt in range(YT):
                    nc.tensor.matmul(nel[0:1, xh * 512:(xh + 1) * 512], ones[:],
                                     MTF[:, yt, xh * 512:(xh + 1) * 512],
                                     start=(yt == 0), stop=(yt == YT - 1))
            nelc = pr.tile([1, LX], F32, tag="nelc", name=f"nelc{b}")
            nc.vector.tensor_scalar(nelc[:], nel[:], 1.0, INV_SCALE, AL.max, AL.mult)
            invr = pr.tile([1, LX], F32, tag="invr", name=f"invr{b}")
            nc.vector.reciprocal(invr[:], nelc[:])
            invb = pr.tile([128, LX], F32, tag="invb", name=f"invb{b}")
            nc.gpsimd.partition_broadcast(invb[:], invr[:])

            # ---- attention ----
            E = pr.tile([128, KB, LX], F16, tag="e", name=f"e{b}")
            for hp in range(HP):
                C = [pc.tile([128, LX], F32, tag=f"c{j}", name=f"c{j}_{b}_{hp}")
                     for j in range(2)]
                for yt in range(YT):
                    A = [pa.tile([128, LX], F32, tag="a", name=f"a{j}_{b}_{hp}_{yt}")
                         for j in range(2)]
                    for j in range(2):
                        hs = slice(64 * j, 64 * (j + 1))
                        for xh in range(XH):
                            nc.tensor.matmul(
                                A[j][:, xh * 512:(xh + 1) * 512],
                                K[hs, hp, yt * 128:(yt + 1) * 128],
                                Q[hs, hp, xh * 512:(xh + 1) * 512],
                                start=True, stop=True)
                    Am = [amp.tile([128, LX], F16, tag=f"am{j}", name=f"am{j}_{b}_{hp}_{yt}")
                          for j in range(2)]
                    for j in range(2):
                        if ((hp * YT + yt) * 2 + j) % 100 < ACT_PCT:
                            At = amp.tile([128, LX], F16, tag="at",
                                          name=f"at_{b}_{hp}_{yt}_{j}")
                            nc.scalar.activation(At[:], A[j][:],
                                                 mybir.ActivationFunctionType.Relu)
                            nc.vector.tensor_tensor(Am[j][:], At[:], MTF[:, yt, :],
                                                    AL.mult)
                        else:
                            nc.vector.scalar_tensor_tensor(
                                Am[j][:], A[j][:], 0.0, MTF[:, yt, :], AL.max, AL.mult)
                    for j in range(2):
                        hs = slice(64 * j, 64 * (j + 1))
                        for xh in range(XH):
                            nc.tensor.matmul(
                                C[j][hs, xh * 512:(xh + 1) * 512],
                                KT[:, yt, hp * 128 + 64 * j: hp * 128 + 64 * (j + 1)],
                                Am[j][:, xh * 512:(xh + 1) * 512],
                                start=(yt == 0), stop=(yt == YT - 1))
                Et = amp.tile([128, LX], F16, tag="et", name=f"et_{b}_{hp}")
                for j in range(2):
                    hs = slice(64 * j, 64 * (j + 1))
                    nc.vector.tensor_tensor(Et[hs, :], C[j][hs, :], invb[hs, :],
                                            AL.mult)
                nc.vector.tensor_tensor(E[:, hp, :], Et[:], Q[:, hp, :], AL.add)

            # ---- output projection ----
            for m in range(KB):
                ps = pa.tile([128, 1024], F32, tag="a", name=f"po{b}_{m}")
                for k in range(KB):
                    for n in range(XH):
                        nc.tensor.matmul(ps[:, n * 512:(n + 1) * 512],
                                         WOT[:, k, m * 128:(m + 1) * 128],
                                         E[:, k, n * 512:(n + 1) * 512],
                                         start=(k == 0), stop=(k == KB - 1))
                oS = osp.tile([128, LX], F32, tag="os", name=f"os{b}_{m}")
                nc.scalar.copy(oS[:], ps[:])
                nc.sync.dma_start(o_e.ap()[b, m * 128:(m + 1) * 128, :], oS[:])


def _get_nc():
    if "nc" not in _CACHE:
        _CACHE["nc"] = _build()
    return _CACHE["nc"]


def kernel(x, y, xy_mask, wq, wk, wo):
    nc = _get_nc()
    xf = x.astype(np.float16)
    yf = y.astype(np.float16)
    mtt = np.ascontiguousarray(
        xy_mask.view(np.uint8).transpose(0, 2, 1))
    wqt = np.ascontiguousarray(wq.T).astype(np.float16)
    wkt = np.ascontiguousarray(wk.T).astype(np.float16)
    wot = np.ascontiguousarray((0.5 * wo).T).astype(np.float16)
    in_maps = [
        {"x": xf[c * BPC:(c + 1) * BPC], "y": yf[c * BPC:(c + 1) * BPC],
         "mt": mtt[c * BPC:(c + 1) * BPC], "wqt": wqt, "wkt": wkt, "wot": wot}
        for c in range(N_CORES)
    ]
    res = run_bass_kernel_spmd(nc, in_maps, list(range(N_CORES)), trace=TRACE)
    if TRACE:
        _CACHE["last_exec_time_ns"] = res.exec_time_ns
        _CACHE["last_profile_json"] = res.profile_json
    return np.concatenate([res.results[c]["o"] for c in range(N_CORES)], axis=0)


# revision 3
# speedup vs baseline: 1.2706x; 1.2706x over previous
"""MultiHeadSimilarity kernel for 8 Trainium2 NeuronCores.

Reference computation (per batch b):
    Q = wq @ x[b];  K = wk @ y[b]                       (channel-mixing matmuls)
    per head h (d=64):  A = relu(Qh^T Kh) * scale, masked by xy_mask
    C = A @ Kh^T, normalized per-row by 1/max(sum(mask, y), 1)
    out = wo @ (0.5 * (Q + C))

Sharding: data-parallel over batch; 16 batches / 8 cores = 2 per core.
Weights replicated. No cross-core communication.

Device algorithm (per core, fp16 compute with fp32 PSUM accumulation):
  - Q = wqT.T @ x, K = wkT.T @ y, KT = y.T @ wkT (natural-layout matmuls; the
    K transpose needed by the C-contraction is computed as a second projection
    instead of an on-chip transpose).
  - A is computed transposed (y on partitions) per head; relu+mask are fused
    into one DVE scalar_tensor_tensor: (A max 0) * maskT, or routed through
    ScalarE relu + DVE multiply when that balances engine load better.
    Two heads are packed in the 128-wide PE array (K=64 row groups for the
    A matmuls / M=64 col groups for the C matmuls).
  - n_el row counts come from a ones^T @ maskT matmul; inv = 1/(8*max(n,1))
    folds the 1/sqrt(d) attention scale; 0.5 is folded into woT on the host.
"""
import sys

if "/opt/trn_rl_repo" not in sys.path:
    sys.path.insert(0, "/opt/trn_rl_repo")

import numpy as np

import concourse.tile as tile
from concourse import bacc, mybir
from concourse.bass_utils import run_bass_kernel_spmd

F16 = mybir.dt.float16
F32 = mybir.dt.float32
AL = mybir.AluOpType
RELU = mybir.ActivationFunctionType.Relu

N_CORES = 8
B, U, LX, LY, H, D = 16, 512, 1024, 1024, 8, 64
BPC = B // N_CORES          # batches per core
KB = U // 128               # 4  k-tiles over channels
HP = H // 2                 # 4  head pairs
YT = LY // 128              # 8  y tiles
XH = LX // 512              # 2  x halves
INV_SCALE = float(D) ** 0.5  # 8.0; attention scale = 1/8

TRACE = False
_CACHE = {}


class Balance:
    """Greedy static load balancer between DVE (vector) and ACT (scalar)."""

    def __init__(self, nc):
        self.nc = nc
        self.t = {"v": 0.0, "s": 0.0}

    def copy(self, dst, src, fd):
        dve = 120 + fd
        act = (172 + fd) * 0.8
        if self.t["v"] + dve <= self.t["s"] + act:
            self.t["v"] += dve
            self.nc.vector.tensor_copy(dst, src)
        else:
            self.t["s"] += act
            self.nc.scalar.copy(dst, src)

    def relu_mask(self, out, a_ps, mtf, tmp_pool, fd, name):
        # option 1: fused on DVE; option 2: ScalarE relu + DVE f16 multiply
        stt = 120 + fd
        act = (172 + fd) * 0.8
        tt = 58 + fd / 2
        if self.t["v"] + stt <= max(self.t["s"] + act, self.t["v"] + tt):
            self.t["v"] += stt
            self.nc.vector.scalar_tensor_tensor(out, a_ps, 0.0, mtf, AL.max, AL.mult)
        else:
            self.t["s"] += act
            self.t["v"] += tt
            at = tmp_pool.tile(list(out.shape), F16, tag="at", name=name)
            self.nc.scalar.activation(at[:], a_ps, RELU)
            self.nc.vector.tensor_tensor(out, at[:], mtf, AL.mult)


def _build():
    nc = bacc.Bacc("TRN2", target_bir_lowering=False, debug=False,
                   num_devices=N_CORES)
    x_e = nc.dram_tensor("x", [BPC, U, LX], F16, kind="ExternalInput")
    y_e = nc.dram_tensor("y", [BPC, U, LY], F16, kind="ExternalInput")
    mt_e = nc.dram_tensor("mt", [BPC, LY, LX], F16, kind="ExternalInput")
    wqt_e = nc.dram_tensor("wqt", [U, U], F16, kind="ExternalInput")
    wkt_e = nc.dram_tensor("wkt", [U, U], F16, kind="ExternalInput")
    wot_e = nc.dram_tensor("wot", [U, U], F16, kind="ExternalInput")
    o_e = nc.dram_tensor("o", [BPC, U, LX], F32, kind="ExternalOutput")

    with tile.TileContext(nc) as tc:
        _emit(nc, tc, x_e, y_e, mt_e, wqt_e, wkt_e, wot_e, o_e)
    nc.compile()
    return nc


def _emit(nc, tc, x_e, y_e, mt_e, wqt_e, wkt_e, wot_e, o_e):
    import contextlib
    bal = Balance(nc)
    ctx = contextlib.ExitStack()
    with ctx:
        wp = ctx.enter_context(tc.tile_pool(name="wp", bufs=1))
        io = ctx.enter_context(tc.tile_pool(name="io", bufs=2))
        pr = ctx.enter_context(tc.tile_pool(name="pr", bufs=2))
        sm = ctx.enter_context(tc.tile_pool(name="sm", bufs=2))
        amp = ctx.enter_context(tc.tile_pool(name="amp", bufs=3))
        osp = ctx.enter_context(tc.tile_pool(name="osp", bufs=4))
        pp = ctx.enter_context(tc.tile_pool(name="pp", bufs=1, space="PSUM"))
        pa = ctx.enter_context(tc.tile_pool(name="pa", bufs=4, space="PSUM"))
        pc = ctx.enter_context(tc.tile_pool(name="pc", bufs=1, space="PSUM"))

        # weights, loaded once
        WQT = wp.tile([128, KB, U], F16, tag="wqt")
        WKT = wp.tile([128, KB, U], F16, tag="wkt")
        WOT = wp.tile([128, KB, U], F16, tag="wot")
        for w_t, w_e in ((WQT, wqt_e), (WKT, wkt_e), (WOT, wot_e)):
            nc.sync.dma_start(w_t[:], w_e.ap().rearrange("(k p) o -> p k o", p=128))
        ones = wp.tile([128, 1], F16, tag="ones")
        nc.vector.memset(ones[:], 1.0)

        for b in range(BPC):
            # ---- input loads ----
            X = io.tile([128, KB, LX], F16, tag="x", name=f"x{b}")
            nc.sync.dma_start(X[:], x_e.ap()[b].rearrange("(k p) l -> p k l", p=128))
            Y = io.tile([128, KB, LY], F16, tag="y", name=f"y{b}")
            nc.sync.dma_start(Y[:], y_e.ap()[b].rearrange("(k p) l -> p k l", p=128))
            MTF = io.tile([128, YT, LX], F16, tag="mtf", name=f"mtf{b}")
            nc.sync.dma_start(MTF[:], mt_e.ap()[b].rearrange("(t p) l -> p t l", p=128))

            # ---- projections ----
            Q = pr.tile([128, KB, LX], F16, tag="q", name=f"q{b}")
            K = pr.tile([128, KB, LY], F16, tag="k", name=f"k{b}")
            for w_t, src, dst in ((WQT, X, Q), (WKT, Y, K)):
                for m in range(KB):
                    ps = pp.tile([128, 1024], F32, tag="p", name=f"pj{b}_{dst.name}_{m}")
                    for k in range(KB):
                        for n in range(XH):
                            nc.tensor.matmul(
                                ps[:, n * 512:(n + 1) * 512],
                                w_t[:, k, m * 128:(m + 1) * 128],
                                src[:, k, n * 512:(n + 1) * 512],
                                start=(k == 0), stop=(k == KB - 1))
                    bal.copy(dst[:, m, :], ps[:], 1024)
            KT = pr.tile([128, YT, U], F16, tag="kt", name=f"kt{b}")
            for lt in range(YT):
                ps = pp.tile([128, 1024], F32, tag="p", name=f"pkt{b}_{lt}")
                for k in range(KB):
                    nc.tensor.matmul(ps[:, 0:512],
                                     Y[:, k, lt * 128:(lt + 1) * 128],
                                     WKT[:, k, :512],
                                     start=(k == 0), stop=(k == KB - 1))
                bal.copy(KT[:, lt, :], ps[:, 0:512], 512)

            # ---- mask row counts and inverse ----
            nel = pp.tile([1, 1024], F32, tag="p", name=f"nel{b}")
            for xh in range(XH):
                for yt in range(YT):
                    nc.tensor.matmul(nel[0:1, xh * 512:(xh + 1) * 512], ones[:],
                                     MTF[:, yt, xh * 512:(xh + 1) * 512],
                                     start=(yt == 0), stop=(yt == YT - 1))
            nelc = sm.tile([1, LX], F32, tag="nelc", name=f"nelc{b}")
            nc.vector.tensor_scalar(nelc[:], nel[:], 1.0, INV_SCALE, AL.max, AL.mult)
            # reciprocal is ~8 cycles/free-element on DVE; bounce through a
            # (128 x 8) layout so the iteration count is 8, not 1024
            nelp = sm.tile([128, 8], F32, tag="nelp", name=f"nelp{b}")
            nc.sync.dma_start(nelp[:], nelc[:])
            invp = sm.tile([128, 8], F32, tag="invp", name=f"invp{b}")
            nc.vector.reciprocal(invp[:], nelp[:])
            invr = sm.tile([1, LX], F32, tag="invr", name=f"invr{b}")
            nc.sync.dma_start(invr[:], invp[:])
            invb = sm.tile([128, LX], F32, tag="invb", name=f"invb{b}")
            nc.gpsimd.partition_broadcast(invb[:], invr[:])

            # ---- attention ----
            E = pr.tile([128, KB, LX], F16, tag="e", name=f"e{b}")
            for hp in range(HP):
                for xh in range(XH):
                    xs = slice(xh * 512, (xh + 1) * 512)
                    C = [pc.tile([128, 512], F32, tag=f"c{j}", name=f"c{j}_{b}_{hp}_{xh}")
                         for j in range(2)]
                    for yt in range(YT):
                        A = [pa.tile([128, 512], F32, tag="a",
                                     name=f"a{j}_{b}_{hp}_{xh}_{yt}")
                             for j in range(2)]
                        for j in range(2):
                            hs = slice(64 * j, 64 * (j + 1))
                            nc.tensor.matmul(
                                A[j][:], K[hs, hp, yt * 128:(yt + 1) * 128],
                                Q[hs, hp, xs], start=True, stop=True)
                        Am = [amp.tile([128, 512], F16, tag=f"am{j}",
                                       name=f"am{j}_{b}_{hp}_{xh}_{yt}")
                              for j in range(2)]
                        for j in range(2):
                            bal.relu_mask(Am[j][:], A[j][:], MTF[:, yt, xs], amp,
                                          512, f"at_{b}_{hp}_{xh}_{yt}_{j}")
                        for j in range(2):
                            hs = slice(64 * j, 64 * (j + 1))
                            nc.tensor.matmul(
                                C[j][hs, :],
                                KT[:, yt, hp * 128 + 64 * j: hp * 128 + 64 * (j + 1)],
                                Am[j][:], start=(yt == 0), stop=(yt == YT - 1))
                    Et = amp.tile([128, 512], F16, tag="et", name=f"et_{b}_{hp}_{xh}")
                    for j in range(2):
                        hs = slice(64 * j, 64 * (j + 1))
                        nc.vector.tensor_tensor(Et[hs, :], C[j][hs, :],
                                                invb[hs, xs], AL.mult)
                    nc.vector.tensor_tensor(E[:, hp, xs], Et[:], Q[:, hp, xs], AL.add)

            # ---- output projection ----
            for m in range(KB):
                ps = pp.tile([128, 1024], F32, tag="p", name=f"po{b}_{m}")
                for k in range(KB):
                    for n in range(XH):
                        nc.tensor.matmul(ps[:, n * 512:(n + 1) * 512],
                                         WOT[:, k, m * 128:(m + 1) * 128],
                                         E[:, k, n * 512:(n + 1) * 512],
                                         start=(k == 0), stop=(k == KB - 1))
                oS = osp.tile([128, LX], F32, tag="os", name=f"os{b}_{m}")
                bal.copy(oS[:], ps[:], 1024)
                nc.sync.dma_start(o_e.ap()[b, m * 128:(m + 1) * 128, :], oS[:])


def _get_nc():
    if "nc" not in _CACHE:
        _CACHE["nc"] = _build()
    return _CACHE["nc"]


def kernel(x, y, xy_mask, wq, wk, wo):
    nc = _get_nc()
    xf = x.astype(np.float16)
    yf = y.astype(np.float16)
    mtt = np.ascontiguousarray(
        xy_mask.transpose(0, 2, 1)).astype(np.float16)
    wqt = np.ascontiguousarray(wq.T).astype(np.float16)
    wkt = np.ascontiguousarray(wk.T).astype(np.float16)
    wot = np.ascontiguousarray((0.5 * wo).T).astype(np.float16)
    in_maps = [
        {"x": xf[c * BPC:(c + 1) * BPC], "y": yf[c * BPC:(c + 1) * BPC],
         "mt": mtt[c * BPC:(c + 1) * BPC], "wqt": wqt, "wkt": wkt, "wot": wot}
        for c in range(N_CORES)
    ]
    res = run_bass_kernel_spmd(nc, in_maps, list(range(N_CORES)), trace=TRACE)
    if TRACE:
        _CACHE["last_exec_time_ns"] = res.exec_time_ns
        _CACHE["last_profile_json"] = res.profile_json
    return np.concatenate([res.results[c]["o"] for c in range(N_CORES)], axis=0)
